# revision 16
# baseline (speedup 1.0000x reference)
"""Trainium2 Bass kernel: causal attention (QKV projection + causal softmax + AV).

Problem: x[4, 4096, 768] fp32, per-head projections to d=64, full causal
attention per batch, output [4, 4096, 64] fp32.

Sharding: 8 cores = 4 batches x 2 parity groups. Core (b, j) computes the
output rows of batch b whose 128-row block index i satisfies i % 2 == j.
One uniform SPMD program: for j=0 cores the host shifts x down by one
128-row block (prepending zeros), which makes the causal structure of both
parities identical in device coordinates (device q-blocks are always the odd
blocks 1,3,...,31; k-slot g holds true block g-1 for j=0 and g for j=1; the
dead slot 0 of j=0 is neutralized by zeroing V' slot 0, so its exp(0)=1
weights contribute nothing to numerator or denominator).

Device pipeline per core:
  x^T arrives host-pre-transposed (plain DMA loads, no DMA-transpose).
  A short stream of dummy matmuls at t=0 keeps the tensor engine
  continuously busy so its p-state clock is fully ramped when real
  projections start.
  Projections per 512-row chunk: stationary [wq] produces Q^T on PSUM
  partitions 64-127; stationary [wv|wk] produces V^T (0-63) and K^T (64-127).
  Q^T (+bq) and K^T (bk dropped: softmax is invariant to score offsets that
  are constant along k) are written as fp8e4 into zero-padded DoubleRow
  buffers [64, 2, cols] whose second contraction-tile group is zeroed once.
  Attention runs as one flat pipeline of slot pairs across all q-chunks
  (chunk 0 split into two 256-column halves to shorten the startup
  dependency), with projection work for later chunks emitted between pairs
  and each pair's AV lagging one pair behind its scores so the tensor
  engine never waits on the exp.
  Scores: one fp8 DoubleRow matmul per k-slot (0.5 cycles/row), the pair's
  slots packed tail/head around the tile midpoint so each pair is a single
  exact-width exp on ACT. The two slots of a pair share width and column
  range, so AV is one DoubleRow matmul per pair over fp8 P and fp8 V'
  (the pair as the two contraction-tile groups, 80-byte slot stride for
  the dual-fp8 16-byte alignment rule), accumulating into a [80, 512] PSUM
  tile whose row 64 is the softmax denominator. Chunk-0a pair-0 (rows that
  attend very few keys, where fp8 V error would not average out) uses bf16
  P/V'. Host divides and transposes.
"""

import numpy as np
import ml_dtypes
from contextlib import ExitStack

import concourse.bass as bass
import concourse.mybir as mybir
import concourse.tile as tile
from concourse import bacc
from concourse.bass_utils import run_bass_kernel_spmd

F32 = mybir.dt.float32
BF16 = mybir.dt.bfloat16
FP8 = mybir.dt.float8e4

SEQ = 4096
DIN = 768
DOUT = 64
NCC = DIN // 128          # 6 contraction chunks
NSC = SEQ // 512          # 8 seq chunks (projection granularity)
NBLK = SEQ // 128         # 32 k-slots
NQC = 4                   # q chunks of 512 local columns (2048 own q rows)
NWARM = 16                # PE p-state warmup matmuls
SCALE = 1.0 / 8.0
EXPF = mybir.ActivationFunctionType.Exp
DR = mybir.MatmulPerfMode.DoubleRow

_CACHED_NC = None


def build_nc(dump=False, repeats=1):
    nc = bacc.Bacc("TRN2", target_bir_lowering=False, debug=False)

    xt = nc.dram_tensor("xt", [DIN, SEQ], BF16, kind="ExternalInput")
    wqr = nc.dram_tensor("wqr", [128, NCC * 64], BF16, kind="ExternalInput")
    wkvr = nc.dram_tensor("wkvr", [128, NCC * 128], BF16, kind="ExternalInput")
    bq = nc.dram_tensor("bq", [64, 1], F32, kind="ExternalInput")
    bv = nc.dram_tensor("bv", [64, 1], F32, kind="ExternalInput")
    pads = nc.dram_tensor("pads", [128, 1], F32, kind="ExternalInput")   # 1 / 0
    maska = nc.dram_tensor("maska", [128, 512], BF16, kind="ExternalInput")
    idnb = nc.dram_tensor("idnb", [64, 64], BF16, kind="ExternalInput")
    o = nc.dram_tensor("o", [NQC, 65, 512], F32, kind="ExternalOutput")

    with tile.TileContext(nc) as tc, ExitStack() as ctx:
        cpool = ctx.enter_context(tc.tile_pool(name="const", bufs=1))
        vtp = ctx.enter_context(tc.tile_pool(name="vt", bufs=2))
        ptp = ctx.enter_context(tc.tile_pool(name="pt", bufs=3))
        ocp = ctx.enter_context(tc.tile_pool(name="oc", bufs=2))
        psproj = ctx.enter_context(tc.tile_pool(name="psproj", bufs=2, space="PSUM"))
        psst = ctx.enter_context(tc.tile_pool(name="psst", bufs=2, space="PSUM"))
        psav = ctx.enter_context(tc.tile_pool(name="psav", bufs=2, space="PSUM"))

        wq_sb = cpool.tile([128, NCC * 64], BF16)
        wkv_sb = cpool.tile([128, NCC * 128], BF16)
        bq_sb = cpool.tile([128, 1], F32)     # rows 64:128 hold bq
        bv_sb = cpool.tile([64, 1], F32)
        pads_sb = cpool.tile([128, 1], F32)
        mask_sb = cpool.tile([128, 512], BF16)
        idn_sb = cpool.tile([64, 64], BF16)
        warm = cpool.tile([128, 256], BF16)
        xtf = cpool.tile([128, NSC * NCC * 512], BF16)  # x^T, [p, (sc, cc, s)]
        qdr = cpool.tile([128, 2 * 2048], FP8)  # Q^T fp8, rows 64:128, [(i, q)]
        kdr = cpool.tile([128, 2 * 4096], FP8)  # K^T fp8, rows 64:128, [(i, m)]
        vs = cpool.tile([128, NBLK * 80], FP8)  # V' = [V | 1 | 0pad] per k-slot
        # (80-wide slots: dual-fp8 ldweights needs a 16-byte-aligned
        # stride between the two contraction-tile groups)
        vsb = cpool.tile([128, 2 * 65], BF16)   # bf16 V' for slots 0,1

        # PE p-state warmup: dummy matmuls on a zeroed tile keep the tensor
        # engine busy from t~0 so the clock is ramped when real work arrives
        nc.vector.memset(warm[:], 0.0)
        for _ in range(NWARM):
            wp = psproj.tile([128, 256], F32, tag="proj")
            nc.tensor.matmul(wp[:], warm[:, 0:128], warm[:], start=True, stop=True)

        # x^T columns arrive host-permuted per 512-chunk: [odd blocks 1,3 |
        # even blocks 0,2]. The odd half feeds passA (own q rows), so it can
        # be loaded first; K/V projections consume the whole permuted chunk
        # (K^T/V' slot bookkeeping maps the permutation back).
        def loadxh(sc, h):
            nc.sync.dma_start(
                xtf[:, sc * NCC * 512:(sc + 1) * NCC * 512]
                .rearrange("p (cc h s) -> p cc h s", cc=NCC, h=2)[:, :, h, :],
                xt.rearrange("(cc p) s -> p cc s", p=128)[
                    :, :, sc * 512 + h * 256:sc * 512 + (h + 1) * 256
                ],
            )

        loadxh(0, 0)
        nc.sync.dma_start(wq_sb[:], wqr[:, :])
        nc.sync.dma_start(bq_sb[64:128, :], bq[:, :])
        nc.sync.dma_start(wkv_sb[:], wkvr[:, :])
        nc.sync.dma_start(bv_sb[:], bv[:, :])
        loadxh(0, 1)
        nc.sync.dma_start(idn_sb[:], idnb[:, :])
        nc.sync.dma_start(mask_sb[:], maska[:, :])
        nc.sync.dma_start(pads_sb[:], pads[:, :])
        loadxh(1, 0)
        loadxh(1, 1)
        loadxh(2, 0)
        loadxh(3, 0)
        loadxh(2, 1)
        loadxh(3, 1)
        loadxh(4, 0)
        loadxh(4, 1)
        loadxh(5, 0)
        loadxh(5, 1)
        loadxh(6, 0)
        loadxh(6, 1)
        loadxh(7, 0)
        loadxh(7, 1)

        # ones column + zero pad of V'
        nc.vector.memset(
            vs[:].rearrange("p (g e) -> p g e", g=NBLK)[:, :, 64:65], 1.0
        )
        nc.vector.memset(
            vs[:].rearrange("p (g e) -> p g e", g=NBLK)[:, :, 65:80], 0.0
        )
        nc.vector.memset(
            vsb[:].rearrange("p (g e) -> p g e", g=2)[:, :, 64:65], 1.0
        )
        # zero the second DoubleRow contraction-tile group of Q^T/K^T (both
        # sides, guarding against NaN garbage multiplying the other's zeros);
        # ordered so the regions attention needs first are zeroed first
        nc.gpsimd.memset(qdr[64:128, 2048:2048 + 512], 0.0)
        nc.gpsimd.memset(kdr[64:128, 4096:4096 + 1024], 0.0)
        nc.gpsimd.memset(kdr[64:128, 4096 + 1024:8192], 0.0)
        nc.gpsimd.memset(qdr[64:128, 2048 + 512:4096], 0.0)

        def xts(sc, cc):
            base = sc * NCC * 512 + cc * 512
            return xtf[:, base:base + 512]

        def passA(sc):
            """Q^T for own (odd) q-blocks of this chunk, fp8 into qdr."""
            qp = psproj.tile([128, 256], F32, tag="proj")
            for cc in range(NCC):
                # odd-block q columns are the first (contiguous) half of the
                # permuted chunk
                nc.tensor.matmul(
                    qp[64:128, :], wq_sb[:, cc * 64:(cc + 1) * 64],
                    xts(sc, cc)[:, 0:256],
                    start=(cc == 0), stop=(cc == NCC - 1),
                )
            nc.vector.tensor_scalar_add(
                qdr[64:128, :].rearrange("p (i q) -> p i q", i=2)[
                    :, 0, sc * 256:(sc + 1) * 256
                ],
                qp[64:128, :], bq_sb[64:128, :],
            )

        vt_pend = {}

        def passB_kv(sc):
            """K^T (fp8, no bias) into kdr; V^T (+bias) into a bf16 staging
            tile (transposed into V' by passB_vt)."""
            kvp = psproj.tile([128, 512], F32, tag="proj")
            for cc in range(NCC):
                nc.tensor.matmul(
                    kvp[:], wkv_sb[:, cc * 128:(cc + 1) * 128],
                    xts(sc, cc),
                    start=(cc == 0), stop=(cc == NCC - 1),
                )
            vt = vtp.tile([128, 512], BF16)
            nc.vector.tensor_scalar_add(
                vt[0:64, :], kvp[0:64, :], bv_sb[:, :]
            )
            nc.vector.tensor_copy(
                kdr[64:128, :].rearrange("p (i m) -> p i m", i=2)[
                    :, 0, sc * 512:(sc + 1) * 512
                ],
                kvp[64:128, :],
            )
            vt_pend[sc] = vt

        def passB_vt(sc):
            """PE-transpose V^T chunk into fp8 V' slots (vp groups are in
            the permuted order [b1, b3, b0, b2]; the copies scatter them
            back to logical slot order)."""
            vt = vt_pend.pop(sc)
            vp = psproj.tile([128, 256], BF16, tag="proj")
            for t in range(4):
                nc.tensor.transpose(
                    vp[:, t * 64:(t + 1) * 64],
                    vt[0:64, t * 128:(t + 1) * 128],
                    idn_sb[:],
                )
            vsg = vs[:].rearrange("p (s j i e) -> p s j i e", s=NSC, j=2, e=80)
            vpg = vp[:].rearrange("p (g e) -> p g e", g=4)
            nc.vector.tensor_copy(vsg[:, sc, :, 1, 0:64], vpg[:, 0:2, :])
            nc.vector.tensor_copy(vsg[:, sc, :, 0, 0:64], vpg[:, 2:4, :])
            if sc == 0:
                vbg = vsb[:].rearrange("p (g e) -> p g e", g=2)
                nc.vector.tensor_copy(vbg[:, 0:1, 0:64], vpg[:, 2:3, :])
                nc.vector.tensor_copy(vbg[:, 1:2, 0:64], vpg[:, 0:1, :])
                # neutralize the j=0 dead slot 0 (pads = 0 there, 1 for j=1)
                nc.vector.tensor_scalar_mul(
                    vs[:, 0:80], vs[:, 0:80], pads_sb[:]
                )
                nc.vector.tensor_scalar_mul(
                    vsb[:, 0:65], vsb[:, 0:65], pads_sb[:]
                )

        def passB(sc):
            passB_kv(sc)
            passB_vt(sc)

        # K^T is stored in the permuted per-chunk column order [b1,b3,b0,b2]
        KPOS = {1: 0, 3: 1, 0: 2, 2: 3}

        def kslot(g):
            base = (g // 4) * 512 + KPOS[g % 4] * 128
            return kdr[64:128, :].rearrange("p (i m) -> p i m", i=2)[
                :, :, base:base + 128
            ]

        def qsl(qlo, w):
            return qdr[64:128, :].rearrange("p (i q) -> p i q", i=2)[
                :, :, qlo:qlo + w
            ]

        # flat pair pipeline: chunk 0 split into 256-col halves 0a/0b, then
        # chunks 1..3. Each pair: two k-slots sharing offset and width.
        #   (key, H, qbase, avw, oslice, pairs[(g0, off, w, masked)])
        chunks = [
            ("0a", 256, 0, 256, (0, 0, 256), [
                (0, 0, 256, True), (2, 128, 128, True)]),
            ("0b", 256, 256, 256, (0, 256, 512), [
                (0, 0, 256, False), (2, 0, 256, False),
                (4, 0, 256, True), (6, 128, 128, True)]),
        ]
        for c in range(1, NQC):
            prs = []
            for p in range(4 * c + 4):
                off = 128 * max(0, p - 4 * c)
                prs.append((2 * p, off, 512 - off, p >= 4 * c))
            chunks.append((str(c), 512, c * 512, 512, (c, 0, 512), prs))

        # global pair index -> projection emitters (PE-stream fillers),
        # placed to match x-chunk DMA arrival order
        fillers = {
            0: [lambda: passA(1)],
            1: [lambda: passB_kv(1)],
            2: [lambda: passB_vt(1)],
            3: [lambda: passA(2)],
            4: [lambda: passA(3)],
            6: [lambda: passB_kv(2)],
            7: [lambda: passB_vt(2)],
            8: [lambda: passB_kv(3)],
            9: [lambda: passB_vt(3)],
            10: [lambda: passA(4)],
            12: [lambda: passA(5)],
            14: [lambda: passB_kv(4)],
            16: [lambda: passB_vt(4)],
            18: [lambda: passB_kv(5)],
            19: [lambda: passB_vt(5)],
            20: [lambda: passA(6)],
            23: [lambda: passA(7)],
            24: [lambda: passB_kv(6)],
            26: [lambda: passB_vt(6)],
            28: [lambda: passB_kv(7)],
            30: [lambda: passB_vt(7)],
        }

        def emit_av(a):
            av_, off, w, pt_, H, pidx, npairs, bf16, out, g0, split = a
            vsl = vs[:].rearrange("p (g e) -> p g e", g=NBLK)[:, g0:g0 + 2, :]
            if bf16:
                nc.tensor.matmul(
                    av_[0:65, 0:256], vsb[:, 0:65], pt_[:, 0:256],
                    start=True, stop=False,
                )
                nc.tensor.matmul(
                    av_[0:65, 0:256], vsb[:, 65:130], pt_[:, 256:512],
                    start=False, stop=False,
                )
            elif split and pidx >= 12:
                # final chunk: column-split pieces so each 128-col output
                # region ships as soon as its last writer lands
                ptg = pt_[:, H - w:H + w].rearrange("p (i q) -> p i q", i=2)
                for r in range(off // 128, 4):
                    qo = 128 * r - off
                    nc.tensor.matmul(
                        av_[:, 128 * r:128 * r + 128],
                        vsl, ptg[:, :, qo:qo + 128],
                        start=False, stop=(pidx == 12 + r),
                        perf_mode=DR, skip_group_check=True,
                    )
                    if pidx == 12 + r:
                        oc = ocp.tile([65, 128], F32)
                        nc.vector.tensor_copy(
                            oc[:], av_[0:65, 128 * r:128 * r + 128]
                        )
                        nc.sync.dma_start(
                            o[NQC - 1, :, 128 * r:128 * (r + 1)], oc[:]
                        )
                return
            else:
                nc.tensor.matmul(
                    av_[:, off:off + w],
                    vsl,
                    pt_[:, H - w:H + w].rearrange("p (i q) -> p i q", i=2),
                    start=(pidx == 0), stop=(not split and pidx == npairs - 1),
                    perf_mode=DR,
                    skip_group_check=split,
                )
            if out is not None:
                c, lo, hi = out
                oc = ocp.tile([65, hi - lo], F32)
                nc.vector.tensor_copy(oc[:], av_[0:65, 0:hi - lo])
                nc.sync.dma_start(o[c, :, lo:hi], oc[:])

        def emit_attention():
            gi = 0
            pend = None
            last_key = chunks[-1][0]
            for key, H, qbase, avw, oslice, prs in chunks:
                av = psav.tile([80, avw], F32, tag="av")
                npairs = len(prs)
                split = key == last_key
                for pidx, (g0, off, w, masked) in enumerate(prs):
                    st = psst.tile([128, 2 * H], F32, tag="st")
                    nc.tensor.matmul(
                        st[:, H - w:H], kslot(g0), qsl(qbase + off, w),
                        start=True, stop=True, perf_mode=DR,
                    )
                    nc.tensor.matmul(
                        st[:, H:H + w], kslot(g0 + 1), qsl(qbase + off, w),
                        start=True, stop=True, perf_mode=DR,
                    )
                    for fn in fillers.get(gi, ()):
                        fn()
                    if pend is not None:
                        emit_av(pend)
                        pend = None
                    bf16 = (key == "0a" and pidx == 0)
                    pt = ptp.tile([128, 2 * H], BF16 if bf16 else FP8)
                    nc.scalar.activation(pt[:, H - w:H + w], st[:, H - w:H + w],
                                         EXPF, bias=0.0, scale=SCALE)
                    if masked:
                        nc.gpsimd.tensor_mul(
                            pt[:, H:H + w], pt[:, H:H + w], mask_sb[:, 0:w]
                        )
                    pend = (av, off, w, pt, H, pidx, npairs, bf16,
                            oslice if (pidx == npairs - 1 and not split)
                            else None, g0, split)
                    gi += 1
            emit_av(pend)

        for _rep in range(repeats):
            passA(0)
            passB(0)
            emit_attention()

    nc.compile()
    return nc


def _get_nc():
    global _CACHED_NC
    if _CACHED_NC is None:
        _CACHED_NC = build_nc()
    return _CACHED_NC


def _host_inputs(x, wq, bq, wk, bk, wv, bv):
    bf = ml_dtypes.bfloat16
    # weights pre-arranged to the on-chip [p, (cc, m)] layout so the DMA
    # moves large contiguous runs
    wqr = np.ascontiguousarray(
        wq.reshape(NCC, 128, 64).transpose(1, 0, 2).reshape(128, NCC * 64)
    ).astype(bf)
    wkv = np.concatenate([wv, wk], axis=1)
    wkvr = np.ascontiguousarray(
        wkv.reshape(NCC, 128, 128).transpose(1, 0, 2).reshape(128, NCC * 128)
    ).astype(bf)
    bqc = bq[:, None].astype(np.float32)
    bvc = bv[:, None].astype(np.float32)
    tri = np.triu(np.ones((128, 128), np.float32))
    maska = np.concatenate([tri, np.ones((128, 384), np.float32)], axis=1).astype(bf)
    idnb = np.eye(64, dtype=np.float32).astype(bf)
    xbf = np.ascontiguousarray(x).astype(bf)

    in_maps = []
    for core in range(8):
        b, j = core // 2, core % 2
        if j == 0:
            xdev = np.concatenate(
                [np.zeros((128, DIN), bf), xbf[b][: SEQ - 128]], axis=0
            )
            ps = np.zeros((128, 1), np.float32)
        else:
            xdev = xbf[b]
            ps = np.ones((128, 1), np.float32)
        # permute each 512-col chunk of x^T to [blocks 1,3 | blocks 0,2]
        xtp = xdev.T.reshape(DIN, NSC, 4, 128)[:, :, [1, 3, 0, 2], :]
        in_maps.append({
            "xt": np.ascontiguousarray(xtp.reshape(DIN, SEQ)),
            "wqr": wqr, "wkvr": wkvr, "bq": bqc, "bv": bvc,
            "pads": ps, "maska": maska, "idnb": idnb,
        })
    return in_maps


def _assemble(results):
    out = np.empty((4, SEQ, DOUT), np.float32)
    for core in range(8):
        b, j = core // 2, core % 2
        od = results[core]["o"]  # [NQC, 65, 512]
        for c in range(NQC):
            num = od[c, 0:64, :].astype(np.float64)
            den = od[c, 64, :].astype(np.float64)
            oc = (num / den).T.astype(np.float32)  # [512, 64]
            for t in range(4):
                r0 = (8 * c + 2 * t + j) * 128
                out[b, r0:r0 + 128] = oc[t * 128:(t + 1) * 128]
    return out


def kernel(x, wq, bq, wk, bk, wv, bv):
    x = np.asarray(x, dtype=np.float32)
    args = [np.asarray(a, dtype=np.float32) for a in (wq, bq, wk, bk, wv, bv)]
    nc = _get_nc()
    in_maps = _host_inputs(x, *args)
    br = run_bass_kernel_spmd(nc, in_maps, core_ids=list(range(8)))
    return _assemble(br.results)


# revision 17
# speedup vs baseline: 1.0455x; 1.0455x over previous
"""Trainium2 Bass kernel: causal attention (QKV projection + causal softmax + AV).

Problem: x[4, 4096, 768] fp32, per-head projections to d=64, full causal
attention per batch, output [4, 4096, 64] fp32.

Sharding: 8 cores = 4 batches x 2 parity groups. Core (b, j) computes the
output rows of batch b whose 128-row block index i satisfies i % 2 == j.
One uniform SPMD program: for j=0 cores the host shifts x down by one
128-row block (prepending zeros), which makes the causal structure of both
parities identical in device coordinates (device q-blocks are always the odd
blocks 1,3,...,31; k-slot g holds true block g-1 for j=0 and g for j=1; the
dead slot 0 of j=0 is neutralized by zeroing V' slot 0, so its exp(0)=1
weights contribute nothing to numerator or denominator).

Device pipeline per core:
  x^T arrives host-pre-transposed (plain DMA loads, no DMA-transpose).
  A short stream of dummy matmuls at t=0 keeps the tensor engine
  continuously busy so its p-state clock is fully ramped when real
  projections start.
  Projections per 512-row chunk: stationary [wq] produces Q^T on PSUM
  partitions 64-127; stationary [wv|wk] produces V^T (0-63) and K^T (64-127).
  Q^T (+bq) and K^T (bk dropped: softmax is invariant to score offsets that
  are constant along k) are written as fp8e4 into zero-padded DoubleRow
  buffers [64, 2, cols] whose second contraction-tile group is zeroed once.
  Attention runs as one flat pipeline of slot pairs across all q-chunks
  (chunk 0 split into two 256-column halves to shorten the startup
  dependency), with projection work for later chunks emitted between pairs
  and each pair's AV lagging one pair behind its scores so the tensor
  engine never waits on the exp.
  Scores: one fp8 DoubleRow matmul per k-slot (0.5 cycles/row), the pair's
  slots packed tail/head around the tile midpoint so each pair is a single
  exact-width exp on ACT. The two slots of a pair share width and column
  range, so AV is one DoubleRow matmul per pair over fp8 P and fp8 V'
  (the pair as the two contraction-tile groups, 80-byte slot stride for
  the dual-fp8 16-byte alignment rule), accumulating into a [80, 512] PSUM
  tile whose row 64 is the softmax denominator. Chunk-0a pair-0 (rows that
  attend very few keys, where fp8 V error would not average out) uses bf16
  P/V'. Host divides and transposes.
"""

import numpy as np
import ml_dtypes
from contextlib import ExitStack

import concourse.bass as bass
import concourse.mybir as mybir
import concourse.tile as tile
from concourse import bacc
from concourse.bass_utils import run_bass_kernel_spmd

F32 = mybir.dt.float32
BF16 = mybir.dt.bfloat16
FP8 = mybir.dt.float8e4

SEQ = 4096
DIN = 768
DOUT = 64
NCC = DIN // 128          # 6 contraction chunks
NSC = SEQ // 512          # 8 seq chunks (projection granularity)
NBLK = SEQ // 128         # 32 k-slots
NQC = 4                   # q chunks of 512 local columns (2048 own q rows)
NWARM = 24                # PE p-state warmup matmuls
SCALE = 1.0 / 8.0
EXPF = mybir.ActivationFunctionType.Exp
DR = mybir.MatmulPerfMode.DoubleRow

_CACHED_NC = None


def build_nc(dump=False, repeats=1):
    nc = bacc.Bacc("TRN2", target_bir_lowering=False, debug=False)

    xt = nc.dram_tensor("xt", [DIN, SEQ], BF16, kind="ExternalInput")
    wcmb = nc.dram_tensor("wcmb", [128, NCC * 192], BF16, kind="ExternalInput")
    bcmb = nc.dram_tensor("bcmb", [128, 2], F32, kind="ExternalInput")
    mcmb = nc.dram_tensor("mcmb", [128, 576], BF16, kind="ExternalInput")
    o = nc.dram_tensor("o", [NQC, 65, 512], F32, kind="ExternalOutput")

    with tile.TileContext(nc) as tc, ExitStack() as ctx:
        cpool = ctx.enter_context(tc.tile_pool(name="const", bufs=1))
        vtp = ctx.enter_context(tc.tile_pool(name="vt", bufs=2))
        ptp = ctx.enter_context(tc.tile_pool(name="pt", bufs=3))
        ocp = ctx.enter_context(tc.tile_pool(name="oc", bufs=2))
        psproj = ctx.enter_context(tc.tile_pool(name="psproj", bufs=2, space="PSUM"))
        psst = ctx.enter_context(tc.tile_pool(name="psst", bufs=2, space="PSUM"))
        psav = ctx.enter_context(tc.tile_pool(name="psav", bufs=2, space="PSUM"))

        w_sb = cpool.tile([128, NCC * 192], BF16)   # [wq cols | wkv cols]
        wq_sb = w_sb[:, 0:NCC * 64]
        wkv_sb = w_sb[:, NCC * 64:]
        b_sb = cpool.tile([128, 2], F32)   # col0 = [bv; bq], col1 = pads
        bq_sb = b_sb[:, 0:1]
        bv_sb = b_sb[0:64, 0:1]
        pads_sb = b_sb[:, 1:2]
        m_sb = cpool.tile([128, 576], BF16)  # [mask | idn]
        mask_sb = m_sb[:, 0:512]
        idn_sb = m_sb[0:64, 512:576]
        warm = cpool.tile([128, 256], BF16)
        xtf = cpool.tile([128, NSC * NCC * 512], BF16)  # x^T, [p, (sc, cc, s)]
        qdr = cpool.tile([128, 2 * 2048], FP8)  # Q^T fp8, rows 64:128, [(i, q)]
        kdr = cpool.tile([128, 2 * 4096], FP8)  # K^T fp8, rows 64:128, [(i, m)]
        vs = cpool.tile([128, NBLK * 80], FP8)  # V' = [V | 1 | 0pad] per k-slot
        # (80-wide slots: dual-fp8 ldweights needs a 16-byte-aligned
        # stride between the two contraction-tile groups)
        vsb = cpool.tile([128, 2 * 65], BF16)   # bf16 V' for slots 0,1

        # PE p-state warmup: dummy matmuls on a zeroed tile keep the tensor
        # engine busy from t~0 so the clock is ramped when real work arrives
        nc.vector.memset(warm[:], 0.0)
        for _ in range(NWARM):
            wp = psproj.tile([128, 256], F32, tag="proj")
            nc.tensor.matmul(wp[:], warm[:, 0:128], warm[:], start=True, stop=True)

        # x^T columns arrive host-permuted per 512-chunk: [odd blocks 1,3 |
        # even blocks 0,2]. The odd half feeds passA (own q rows), so it can
        # be loaded first; K/V projections consume the whole permuted chunk
        # (K^T/V' slot bookkeeping maps the permutation back).
        def loadxh(sc, h):
            nc.sync.dma_start(
                xtf[:, sc * NCC * 512:(sc + 1) * NCC * 512]
                .rearrange("p (cc h s) -> p cc h s", cc=NCC, h=2)[:, :, h, :],
                xt.rearrange("(cc p) s -> p cc s", p=128)[
                    :, :, sc * 512 + h * 256:sc * 512 + (h + 1) * 256
                ],
            )

        loadxh(0, 0)
        nc.sync.dma_start(w_sb[:], wcmb[:, :])
        nc.sync.dma_start(b_sb[:], bcmb[:, :])
        loadxh(0, 1)
        nc.sync.dma_start(m_sb[:], mcmb[:, :])
        loadxh(1, 0)
        loadxh(1, 1)
        loadxh(2, 0)
        loadxh(3, 0)
        loadxh(2, 1)
        loadxh(3, 1)
        loadxh(4, 0)
        loadxh(4, 1)
        loadxh(5, 0)
        loadxh(5, 1)
        loadxh(6, 0)
        loadxh(6, 1)
        loadxh(7, 0)
        loadxh(7, 1)

        # ones column + zero pad of V'
        nc.vector.memset(
            vs[:].rearrange("p (g e) -> p g e", g=NBLK)[:, :, 64:65], 1.0
        )
        nc.vector.memset(
            vs[:].rearrange("p (g e) -> p g e", g=NBLK)[:, :, 65:80], 0.0
        )
        nc.vector.memset(
            vsb[:].rearrange("p (g e) -> p g e", g=2)[:, :, 64:65], 1.0
        )
        # zero the second DoubleRow contraction-tile group of Q^T/K^T (both
        # sides, guarding against NaN garbage multiplying the other's zeros);
        # ordered so the regions attention needs first are zeroed first
        nc.gpsimd.memset(qdr[64:128, 2048:2048 + 512], 0.0)
        nc.gpsimd.memset(kdr[64:128, 4096:4096 + 1024], 0.0)
        nc.gpsimd.memset(kdr[64:128, 4096 + 1024:8192], 0.0)
        nc.gpsimd.memset(qdr[64:128, 2048 + 512:4096], 0.0)

        def xts(sc, cc):
            base = sc * NCC * 512 + cc * 512
            return xtf[:, base:base + 512]

        def passA(sc):
            """Q^T for own (odd) q-blocks of this chunk, fp8 into qdr."""
            qp = psproj.tile([128, 256], F32, tag="proj")
            for cc in range(NCC):
                # odd-block q columns are the first (contiguous) half of the
                # permuted chunk
                nc.tensor.matmul(
                    qp[64:128, :], wq_sb[:, cc * 64:(cc + 1) * 64],
                    xts(sc, cc)[:, 0:256],
                    start=(cc == 0), stop=(cc == NCC - 1),
                )
            nc.vector.tensor_scalar_add(
                qdr[64:128, :].rearrange("p (i q) -> p i q", i=2)[
                    :, 0, sc * 256:(sc + 1) * 256
                ],
                qp[64:128, :], bq_sb[64:128, :],
            )

        vt_pend = {}

        def passB_kv(sc):
            """K^T (fp8, no bias) into kdr; V^T (+bias) into a bf16 staging
            tile (transposed into V' by passB_vt)."""
            kvp = psproj.tile([128, 512], F32, tag="proj")
            for cc in range(NCC):
                nc.tensor.matmul(
                    kvp[:], wkv_sb[:, cc * 128:(cc + 1) * 128],
                    xts(sc, cc),
                    start=(cc == 0), stop=(cc == NCC - 1),
                )
            vt = vtp.tile([128, 512], BF16)
            nc.vector.tensor_scalar_add(
                vt[0:64, :], kvp[0:64, :], bv_sb[:, :]
            )
            nc.vector.tensor_copy(
                kdr[64:128, :].rearrange("p (i m) -> p i m", i=2)[
                    :, 0, sc * 512:(sc + 1) * 512
                ],
                kvp[64:128, :],
            )
            vt_pend[sc] = vt

        def passB_vt(sc):
            """PE-transpose V^T chunk into fp8 V' slots (vp groups are in
            the permuted order [b1, b3, b0, b2]; the copies scatter them
            back to logical slot order)."""
            vt = vt_pend.pop(sc)
            vp = psproj.tile([128, 256], BF16, tag="proj")
            for t in range(4):
                nc.tensor.transpose(
                    vp[:, t * 64:(t + 1) * 64],
                    vt[0:64, t * 128:(t + 1) * 128],
                    idn_sb[:],
                )
            vsg = vs[:].rearrange("p (s j i e) -> p s j i e", s=NSC, j=2, e=80)
            vpg = vp[:].rearrange("p (g e) -> p g e", g=4)
            nc.vector.tensor_copy(vsg[:, sc, :, 1, 0:64], vpg[:, 0:2, :])
            nc.vector.tensor_copy(vsg[:, sc, :, 0, 0:64], vpg[:, 2:4, :])
            if sc == 0:
                vbg = vsb[:].rearrange("p (g e) -> p g e", g=2)
                nc.vector.tensor_copy(vbg[:, 0:1, 0:64], vpg[:, 2:3, :])
                nc.vector.tensor_copy(vbg[:, 1:2, 0:64], vpg[:, 0:1, :])
                # neutralize the j=0 dead slot 0 (pads = 0 there, 1 for j=1)
                nc.vector.tensor_scalar_mul(
                    vs[:, 0:80], vs[:, 0:80], pads_sb[:]
                )
                nc.vector.tensor_scalar_mul(
                    vsb[:, 0:65], vsb[:, 0:65], pads_sb[:]
                )

        def passB(sc):
            passB_kv(sc)
            passB_vt(sc)

        # K^T is stored in the permuted per-chunk column order [b1,b3,b0,b2]
        KPOS = {1: 0, 3: 1, 0: 2, 2: 3}

        def kslot(g):
            base = (g // 4) * 512 + KPOS[g % 4] * 128
            return kdr[64:128, :].rearrange("p (i m) -> p i m", i=2)[
                :, :, base:base + 128
            ]

        def qsl(qlo, w):
            return qdr[64:128, :].rearrange("p (i q) -> p i q", i=2)[
                :, :, qlo:qlo + w
            ]

        # flat pair pipeline: chunk 0 split into 256-col halves 0a/0b, then
        # chunks 1..3. Each pair: two k-slots sharing offset and width.
        #   (key, H, qbase, avw, oslice, pairs[(g0, off, w, masked)])
        chunks = [
            ("0a", 256, 0, 256, (0, 0, 256), [
                (0, 0, 256, True), (2, 128, 128, True)]),
            ("0b", 256, 256, 256, (0, 256, 512), [
                (0, 0, 256, False), (2, 0, 256, False),
                (4, 0, 256, True), (6, 128, 128, True)]),
        ]
        for c in range(1, NQC):
            prs = []
            for p in range(4 * c + 4):
                off = 128 * max(0, p - 4 * c)
                prs.append((2 * p, off, 512 - off, p >= 4 * c))
            chunks.append((str(c), 512, c * 512, 512, (c, 0, 512), prs))

        # global pair index -> projection emitters (PE-stream fillers),
        # placed to match x-chunk DMA arrival order
        fillers = {
            0: [lambda: passA(1)],
            1: [lambda: passB_kv(1)],
            2: [lambda: passB_vt(1)],
            3: [lambda: passA(2)],
            4: [lambda: passA(3)],
            6: [lambda: passB_kv(2)],
            7: [lambda: passB_vt(2)],
            8: [lambda: passB_kv(3)],
            9: [lambda: passB_vt(3)],
            10: [lambda: passA(4)],
            12: [lambda: passA(5)],
            14: [lambda: passB_kv(4)],
            16: [lambda: passB_vt(4)],
            18: [lambda: passB_kv(5)],
            19: [lambda: passB_vt(5)],
            20: [lambda: passA(6)],
            23: [lambda: passA(7)],
            24: [lambda: passB_kv(6)],
            26: [lambda: passB_vt(6)],
            28: [lambda: passB_kv(7)],
            30: [lambda: passB_vt(7)],
        }

        def emit_av(a):
            av_, off, w, pt_, H, pidx, npairs, bf16, out, g0, split = a
            vsl = vs[:].rearrange("p (g e) -> p g e", g=NBLK)[:, g0:g0 + 2, :]
            if bf16:
                nc.tensor.matmul(
                    av_[0:65, 0:256], vsb[:, 0:65], pt_[:, 0:256],
                    start=True, stop=False,
                )
                nc.tensor.matmul(
                    av_[0:65, 0:256], vsb[:, 65:130], pt_[:, 256:512],
                    start=False, stop=False,
                )
            elif split and pidx >= 12:
                # final chunk: column-split pieces so each 128-col output
                # region ships as soon as its last writer lands
                ptg = pt_[:, H - w:H + w].rearrange("p (i q) -> p i q", i=2)
                for r in range(off // 128, 4):
                    qo = 128 * r - off
                    nc.tensor.matmul(
                        av_[:, 128 * r:128 * r + 128],
                        vsl, ptg[:, :, qo:qo + 128],
                        start=False, stop=(pidx == 12 + r),
                        perf_mode=DR, skip_group_check=True,
                    )
                    if pidx == 12 + r:
                        oc = ocp.tile([65, 128], F32)
                        nc.vector.tensor_copy(
                            oc[:], av_[0:65, 128 * r:128 * r + 128]
                        )
                        nc.sync.dma_start(
                            o[NQC - 1, :, 128 * r:128 * (r + 1)], oc[:]
                        )
                return
            else:
                nc.tensor.matmul(
                    av_[:, off:off + w],
                    vsl,
                    pt_[:, H - w:H + w].rearrange("p (i q) -> p i q", i=2),
                    start=(pidx == 0), stop=(not split and pidx == npairs - 1),
                    perf_mode=DR,
                    skip_group_check=split,
                )
            if out is not None:
                c, lo, hi = out
                oc = ocp.tile([65, hi - lo], F32)
                nc.vector.tensor_copy(oc[:], av_[0:65, 0:hi - lo])
                nc.sync.dma_start(o[c, :, lo:hi], oc[:])

        def emit_attention():
            gi = 0
            pend = None
            last_key = chunks[-1][0]
            for key, H, qbase, avw, oslice, prs in chunks:
                av = psav.tile([80, avw], F32, tag="av")
                npairs = len(prs)
                split = key == last_key
                for pidx, (g0, off, w, masked) in enumerate(prs):
                    st = psst.tile([128, 2 * H], F32, tag="st")
                    nc.tensor.matmul(
                        st[:, H - w:H], kslot(g0), qsl(qbase + off, w),
                        start=True, stop=True, perf_mode=DR,
                    )
                    nc.tensor.matmul(
                        st[:, H:H + w], kslot(g0 + 1), qsl(qbase + off, w),
                        start=True, stop=True, perf_mode=DR,
                    )
                    for fn in fillers.get(gi, ()):
                        fn()
                    if pend is not None:
                        emit_av(pend)
                        pend = None
                    bf16 = (key == "0a" and pidx == 0)
                    pt = ptp.tile([128, 2 * H], BF16 if bf16 else FP8)
                    nc.scalar.activation(pt[:, H - w:H + w], st[:, H - w:H + w],
                                         EXPF, bias=0.0, scale=SCALE)
                    if masked:
                        nc.vector.tensor_mul(
                            pt[:, H:H + w], pt[:, H:H + w], mask_sb[:, 0:w]
                        )
                    pend = (av, off, w, pt, H, pidx, npairs, bf16,
                            oslice if (pidx == npairs - 1 and not split)
                            else None, g0, split)
                    gi += 1
            emit_av(pend)

        for _rep in range(repeats):
            passA(0)
            passB(0)
            emit_attention()

    nc.compile()
    return nc


def _get_nc():
    global _CACHED_NC
    if _CACHED_NC is None:
        _CACHED_NC = build_nc()
    return _CACHED_NC


def _host_inputs(x, wq, bq, wk, bk, wv, bv):
    bf = ml_dtypes.bfloat16
    # weights pre-arranged to the on-chip [p, (cc, m)] layout so the DMA
    # moves large contiguous runs
    wqr = wq.reshape(NCC, 128, 64).transpose(1, 0, 2).reshape(128, NCC * 64)
    wkv = np.concatenate([wv, wk], axis=1)
    wkvr = wkv.reshape(NCC, 128, 128).transpose(1, 0, 2).reshape(128, NCC * 128)
    wcmb = np.ascontiguousarray(
        np.concatenate([wqr, wkvr], axis=1)).astype(bf)
    tri = np.triu(np.ones((128, 128), np.float32))
    maska = np.concatenate([tri, np.ones((128, 384), np.float32)], axis=1)
    idnb = np.concatenate(
        [np.eye(64, dtype=np.float32), np.zeros((64, 64), np.float32)], axis=0)
    mcmb = np.ascontiguousarray(
        np.concatenate([maska, idnb], axis=1)).astype(bf)
    bcol = np.concatenate([bv, bq])[:, None].astype(np.float32)
    xbf = np.ascontiguousarray(x).astype(bf)

    in_maps = []
    for core in range(8):
        b, j = core // 2, core % 2
        if j == 0:
            xdev = np.concatenate(
                [np.zeros((128, DIN), bf), xbf[b][: SEQ - 128]], axis=0
            )
            ps = np.zeros((128, 1), np.float32)
        else:
            xdev = xbf[b]
            ps = np.ones((128, 1), np.float32)
        # permute each 512-col chunk of x^T to [blocks 1,3 | blocks 0,2]
        xtp = xdev.T.reshape(DIN, NSC, 4, 128)[:, :, [1, 3, 0, 2], :]
        bcmb = np.concatenate([bcol, ps], axis=1).astype(np.float32)
        in_maps.append({
            "xt": np.ascontiguousarray(xtp.reshape(DIN, SEQ)),
            "wcmb": wcmb, "bcmb": np.ascontiguousarray(bcmb),
            "mcmb": mcmb,
        })
    return in_maps


def _assemble(results):
    out = np.empty((4, SEQ, DOUT), np.float32)
    for core in range(8):
        b, j = core // 2, core % 2
        od = results[core]["o"]  # [NQC, 65, 512]
        for c in range(NQC):
            num = od[c, 0:64, :].astype(np.float64)
            den = od[c, 64, :].astype(np.float64)
            oc = (num / den).T.astype(np.float32)  # [512, 64]
            for t in range(4):
                r0 = (8 * c + 2 * t + j) * 128
                out[b, r0:r0 + 128] = oc[t * 128:(t + 1) * 128]
    return out


def kernel(x, wq, bq, wk, bk, wv, bv):
    x = np.asarray(x, dtype=np.float32)
    args = [np.asarray(a, dtype=np.float32) for a in (wq, bq, wk, bk, wv, bv)]
    nc = _get_nc()
    in_maps = _host_inputs(x, *args)
    br = run_bass_kernel_spmd(nc, in_maps, core_ids=list(range(8)))
    return _assemble(br.results)


# revision 18
# speedup vs baseline: 1.0613x; 1.0151x over previous
"""Trainium2 Bass kernel: causal attention (QKV projection + causal softmax + AV).

Problem: x[4, 4096, 768] fp32, per-head projections to d=64, full causal
attention per batch, output [4, 4096, 64] fp32.

Sharding: 8 cores = 4 batches x 2 parity groups. Core (b, j) computes the
output rows of batch b whose 128-row block index i satisfies i % 2 == j.
One uniform SPMD program: for j=0 cores the host shifts x down by one
128-row block (prepending zeros), which makes the causal structure of both
parities identical in device coordinates (device q-blocks are always the odd
blocks 1,3,...,31; k-slot g holds true block g-1 for j=0 and g for j=1; the
dead slot 0 of j=0 is neutralized by zeroing V' slot 0, so its exp(0)=1
weights contribute nothing to numerator or denominator).

Device pipeline per core:
  x^T arrives host-pre-transposed (plain DMA loads, no DMA-transpose).
  A short stream of dummy matmuls at t=0 keeps the tensor engine
  continuously busy so its p-state clock is fully ramped when real
  projections start.
  Projections per 512-row chunk: stationary [wq] produces Q^T on PSUM
  partitions 64-127; stationary [wv|wk] produces V^T (0-63) and K^T (64-127).
  Q^T (+bq) and K^T (bk dropped: softmax is invariant to score offsets that
  are constant along k) are written as fp8e4 into zero-padded DoubleRow
  buffers [64, 2, cols] whose second contraction-tile group is zeroed once.
  Attention runs as one flat pipeline of slot pairs across all q-chunks
  (chunk 0 split into two 256-column halves to shorten the startup
  dependency), with projection work for later chunks emitted between pairs
  and each pair's AV lagging one pair behind its scores so the tensor
  engine never waits on the exp.
  Scores: one fp8 DoubleRow matmul per k-slot (0.5 cycles/row), the pair's
  slots packed tail/head around the tile midpoint so each pair is a single
  exact-width exp on ACT. The two slots of a pair share width and column
  range, so AV is one DoubleRow matmul per pair over fp8 P and fp8 V'
  (the pair as the two contraction-tile groups, 80-byte slot stride for
  the dual-fp8 16-byte alignment rule), accumulating into a [80, 512] PSUM
  tile whose row 64 is the softmax denominator. Chunk-0a pair-0 (rows that
  attend very few keys, where fp8 V error would not average out) uses bf16
  P/V'. Host divides and transposes.
"""

import numpy as np
import ml_dtypes
from contextlib import ExitStack

import concourse.bass as bass
import concourse.mybir as mybir
import concourse.tile as tile
from concourse import bacc
from concourse.bass_utils import run_bass_kernel_spmd

F32 = mybir.dt.float32
BF16 = mybir.dt.bfloat16
FP8 = mybir.dt.float8e4

SEQ = 4096
DIN = 768
DOUT = 64
NCC = DIN // 128          # 6 contraction chunks
NSC = SEQ // 512          # 8 seq chunks (projection granularity)
NBLK = SEQ // 128         # 32 k-slots
NQC = 4                   # q chunks of 512 local columns (2048 own q rows)
NWARM = 24                # PE p-state warmup matmuls
SCALE = 1.0 / 8.0
EXPF = mybir.ActivationFunctionType.Exp
DR = mybir.MatmulPerfMode.DoubleRow

_CACHED_NC = None


def build_nc(dump=False, repeats=1):
    nc = bacc.Bacc("TRN2", target_bir_lowering=False, debug=False)

    xt = nc.dram_tensor("xt", [DIN, SEQ], BF16, kind="ExternalInput")
    wcmb = nc.dram_tensor("wcmb", [128, NCC * 192], BF16, kind="ExternalInput")
    bcmb = nc.dram_tensor("bcmb", [128, 2], F32, kind="ExternalInput")
    mcmb = nc.dram_tensor("mcmb", [128, 576], BF16, kind="ExternalInput")
    o = nc.dram_tensor("o", [NQC, 65, 512], F32, kind="ExternalOutput")

    with tile.TileContext(nc) as tc, ExitStack() as ctx:
        cpool = ctx.enter_context(tc.tile_pool(name="const", bufs=1))
        vtp = ctx.enter_context(tc.tile_pool(name="vt", bufs=2))
        ptp = ctx.enter_context(tc.tile_pool(name="pt", bufs=3))
        ocp = ctx.enter_context(tc.tile_pool(name="oc", bufs=4))
        psproj = ctx.enter_context(tc.tile_pool(name="psproj", bufs=2, space="PSUM"))
        psst = ctx.enter_context(tc.tile_pool(name="psst", bufs=2, space="PSUM"))
        psav = ctx.enter_context(tc.tile_pool(name="psav", bufs=2, space="PSUM"))

        w_sb = cpool.tile([128, NCC * 192], BF16)   # [wq cols | wkv cols]
        wq_sb = w_sb[:, 0:NCC * 64]
        wkv_sb = w_sb[:, NCC * 64:]
        b_sb = cpool.tile([128, 2], F32)   # col0 = [bv; bq], col1 = pads
        bq_sb = b_sb[:, 0:1]
        bv_sb = b_sb[0:64, 0:1]
        pads_sb = b_sb[:, 1:2]
        m_sb = cpool.tile([128, 576], BF16)  # [mask | idn]
        mask_sb = m_sb[:, 0:512]
        idn_sb = m_sb[0:64, 512:576]
        warm = cpool.tile([128, 256], BF16)
        xtf = cpool.tile([128, NSC * NCC * 512], BF16)  # x^T, [p, (sc, cc, s)]
        qdr = cpool.tile([128, 2 * 2048], FP8)  # Q^T fp8, rows 64:128, [(i, q)]
        kdr = cpool.tile([128, 2 * 4096], FP8)  # K^T fp8, rows 64:128, [(i, m)]
        vs = cpool.tile([128, NBLK * 80], FP8)  # V' = [V | 1 | 0pad] per k-slot
        # (80-wide slots: dual-fp8 ldweights needs a 16-byte-aligned
        # stride between the two contraction-tile groups)
        vsb = cpool.tile([128, 2 * 65], BF16)   # bf16 V' for slots 0,1

        # PE p-state warmup: dummy matmuls on a zeroed tile keep the tensor
        # engine busy from t~0 so the clock is ramped when real work arrives
        nc.vector.memset(warm[:], 0.0)
        for _ in range(NWARM):
            wp = psproj.tile([128, 256], F32, tag="proj")
            nc.tensor.matmul(wp[:], warm[:, 0:128], warm[:], start=True, stop=True)

        # x^T columns arrive host-permuted per 512-chunk: [odd blocks 1,3 |
        # even blocks 0,2]. The odd half feeds passA (own q rows), so it can
        # be loaded first; K/V projections consume the whole permuted chunk
        # (K^T/V' slot bookkeeping maps the permutation back).
        def loadxh(sc, h, cc0=0, cc1=NCC):
            nc.sync.dma_start(
                xtf[:, sc * NCC * 512:(sc + 1) * NCC * 512]
                .rearrange("p (cc h s) -> p cc h s", cc=NCC, h=2)[
                    :, cc0:cc1, h, :
                ],
                xt.rearrange("(cc p) s -> p cc s", p=128)[
                    :, cc0:cc1, sc * 512 + h * 256:sc * 512 + (h + 1) * 256
                ],
            )

        loadxh(0, 0)
        nc.sync.dma_start(w_sb[:], wcmb[:, :])
        loadxh(0, 1, 0, 3)
        nc.sync.dma_start(b_sb[:], bcmb[:, :])
        loadxh(0, 1, 3, 6)
        nc.sync.dma_start(m_sb[:], mcmb[:, :])
        loadxh(1, 0)
        loadxh(1, 1)
        loadxh(2, 0)
        loadxh(3, 0)
        loadxh(2, 1)
        loadxh(3, 1)
        loadxh(4, 0)
        loadxh(4, 1)
        loadxh(5, 0)
        loadxh(5, 1)
        loadxh(6, 0)
        loadxh(6, 1)
        loadxh(7, 0)
        loadxh(7, 1)

        # ones column + zero pad of V'
        nc.vector.memset(
            vs[:].rearrange("p (g e) -> p g e", g=NBLK)[:, :, 64:65], 1.0
        )
        nc.vector.memset(
            vs[:].rearrange("p (g e) -> p g e", g=NBLK)[:, :, 65:80], 0.0
        )
        nc.vector.memset(
            vsb[:].rearrange("p (g e) -> p g e", g=2)[:, :, 64:65], 1.0
        )
        # zero the second DoubleRow contraction-tile group of Q^T/K^T (both
        # sides, guarding against NaN garbage multiplying the other's zeros);
        # ordered so the regions attention needs first are zeroed first
        nc.gpsimd.memset(qdr[64:128, 2048:2048 + 512], 0.0)
        nc.gpsimd.memset(kdr[64:128, 4096:4096 + 1024], 0.0)
        nc.gpsimd.memset(kdr[64:128, 4096 + 1024:8192], 0.0)
        nc.gpsimd.memset(qdr[64:128, 2048 + 512:4096], 0.0)

        def xts(sc, cc):
            base = sc * NCC * 512 + cc * 512
            return xtf[:, base:base + 512]

        def passA(sc):
            """Q^T for own (odd) q-blocks of this chunk, fp8 into qdr."""
            qp = psproj.tile([128, 256], F32, tag="proj")
            for cc in range(NCC):
                # odd-block q columns are the first (contiguous) half of the
                # permuted chunk
                nc.tensor.matmul(
                    qp[64:128, :], wq_sb[:, cc * 64:(cc + 1) * 64],
                    xts(sc, cc)[:, 0:256],
                    start=(cc == 0), stop=(cc == NCC - 1),
                )
            nc.vector.tensor_scalar_add(
                qdr[64:128, :].rearrange("p (i q) -> p i q", i=2)[
                    :, 0, sc * 256:(sc + 1) * 256
                ],
                qp[64:128, :], bq_sb[64:128, :],
            )

        vt_pend = {}

        def passB_kv(sc):
            """K^T (fp8, no bias) into kdr; V^T (+bias) into a bf16 staging
            tile (transposed into V' by passB_vt)."""
            kvp = psproj.tile([128, 512], F32, tag="proj")
            for cc in range(NCC):
                nc.tensor.matmul(
                    kvp[:], wkv_sb[:, cc * 128:(cc + 1) * 128],
                    xts(sc, cc),
                    start=(cc == 0), stop=(cc == NCC - 1),
                )
            nc.vector.tensor_copy(
                kdr[64:128, :].rearrange("p (i m) -> p i m", i=2)[
                    :, 0, sc * 512:(sc + 1) * 512
                ],
                kvp[64:128, :],
            )
            vt = vtp.tile([128, 512], BF16)
            nc.gpsimd.tensor_scalar_add(
                vt[0:64, :], kvp[0:64, :], bv_sb[:, :]
            )
            vt_pend[sc] = vt

        def passB_vt(sc):
            """PE-transpose V^T chunk into fp8 V' slots (vp groups are in
            the permuted order [b1, b3, b0, b2]; the copies scatter them
            back to logical slot order)."""
            vt = vt_pend.pop(sc)
            vp = psproj.tile([128, 256], BF16, tag="proj")
            for t in range(4):
                nc.tensor.transpose(
                    vp[:, t * 64:(t + 1) * 64],
                    vt[0:64, t * 128:(t + 1) * 128],
                    idn_sb[:],
                )
            vsg = vs[:].rearrange("p (s j i e) -> p s j i e", s=NSC, j=2, e=80)
            vpg = vp[:].rearrange("p (g e) -> p g e", g=4)
            nc.vector.tensor_copy(vsg[:, sc, :, 1, 0:64], vpg[:, 0:2, :])
            nc.vector.tensor_copy(vsg[:, sc, :, 0, 0:64], vpg[:, 2:4, :])
            if sc == 0:
                vbg = vsb[:].rearrange("p (g e) -> p g e", g=2)
                nc.vector.tensor_copy(vbg[:, 0:1, 0:64], vpg[:, 2:3, :])
                nc.vector.tensor_copy(vbg[:, 1:2, 0:64], vpg[:, 0:1, :])
                # neutralize the j=0 dead slot 0 (pads = 0 there, 1 for j=1)
                nc.vector.tensor_scalar_mul(
                    vs[:, 0:80], vs[:, 0:80], pads_sb[:]
                )
                nc.vector.tensor_scalar_mul(
                    vsb[:, 0:65], vsb[:, 0:65], pads_sb[:]
                )

        def passB(sc):
            passB_kv(sc)
            passB_vt(sc)

        # K^T is stored in the permuted per-chunk column order [b1,b3,b0,b2]
        KPOS = {1: 0, 3: 1, 0: 2, 2: 3}

        def kslot(g):
            base = (g // 4) * 512 + KPOS[g % 4] * 128
            return kdr[64:128, :].rearrange("p (i m) -> p i m", i=2)[
                :, :, base:base + 128
            ]

        def qsl(qlo, w):
            return qdr[64:128, :].rearrange("p (i q) -> p i q", i=2)[
                :, :, qlo:qlo + w
            ]

        # flat pair pipeline: chunk 0 split into 256-col halves 0a/0b, then
        # chunks 1..3. Each pair: two k-slots sharing offset and width.
        #   (key, H, qbase, avw, oslice, pairs[(g0, off, w, masked)])
        chunks = [
            ("0a", 256, 0, 256, (0, 0, 256), [
                (0, 0, 256, True), (2, 128, 128, True)]),
            ("0b", 256, 256, 256, (0, 256, 512), [
                (0, 0, 256, False), (2, 0, 256, False),
                (4, 0, 256, True), (6, 128, 128, True)]),
        ]
        for c in range(1, NQC):
            prs = []
            for p in range(4 * c + 4):
                off = 128 * max(0, p - 4 * c)
                prs.append((2 * p, off, 512 - off, p >= 4 * c))
            chunks.append((str(c), 512, c * 512, 512, (c, 0, 512), prs))

        # global pair index -> projection emitters (PE-stream fillers),
        # placed to match x-chunk DMA arrival order
        fillers = {
            0: [lambda: passA(1)],
            1: [lambda: passB_vt(0), lambda: passB_kv(1)],
            2: [lambda: passB_vt(1)],
            3: [lambda: passA(2)],
            4: [lambda: passA(3)],
            6: [lambda: passB_kv(2)],
            7: [lambda: passB_vt(2)],
            8: [lambda: passB_kv(3)],
            9: [lambda: passB_vt(3)],
            10: [lambda: passA(4)],
            12: [lambda: passA(5)],
            14: [lambda: passB_kv(4)],
            16: [lambda: passB_vt(4)],
            18: [lambda: passB_kv(5)],
            19: [lambda: passB_vt(5)],
            20: [lambda: passA(6)],
            23: [lambda: passA(7)],
            24: [lambda: passB_kv(6)],
            26: [lambda: passB_vt(6)],
            28: [lambda: passB_kv(7)],
            30: [lambda: passB_vt(7)],
        }

        def emit_av(a):
            av_, off, w, pt_, H, pidx, npairs, bf16, out, g0, split = a
            vsl = vs[:].rearrange("p (g e) -> p g e", g=NBLK)[:, g0:g0 + 2, :]
            if bf16:
                nc.tensor.matmul(
                    av_[0:65, 0:256], vsb[:, 0:65], pt_[:, 0:256],
                    start=True, stop=False,
                )
                nc.tensor.matmul(
                    av_[0:65, 0:256], vsb[:, 65:130], pt_[:, 256:512],
                    start=False, stop=False,
                )
            elif split and pidx >= 12:
                # final chunk: column-split pieces so each 128-col output
                # region ships as soon as its last writer lands
                ptg = pt_[:, H - w:H + w].rearrange("p (i q) -> p i q", i=2)
                for r in range(off // 128, 4):
                    qo = 128 * r - off
                    nc.tensor.matmul(
                        av_[:, 128 * r:128 * r + 128],
                        vsl, ptg[:, :, qo:qo + 128],
                        start=False, stop=(pidx == 12 + r),
                        perf_mode=DR, skip_group_check=True,
                    )
                    if pidx == 12 + r:
                        oc = ocp.tile([65, 128], F32)
                        nc.vector.tensor_copy(
                            oc[:], av_[0:65, 128 * r:128 * r + 128]
                        )
                        nc.sync.dma_start(
                            o[NQC - 1, :, 128 * r:128 * (r + 1)], oc[:]
                        )
                return
            else:
                nc.tensor.matmul(
                    av_[:, off:off + w],
                    vsl,
                    pt_[:, H - w:H + w].rearrange("p (i q) -> p i q", i=2),
                    start=(pidx == 0), stop=(not split and pidx == npairs - 1),
                    perf_mode=DR,
                    skip_group_check=split,
                )
            if out is not None:
                c, lo, hi = out
                oc = ocp.tile([65, hi - lo], F32)
                nc.vector.tensor_copy(oc[:], av_[0:65, 0:hi - lo])
                nc.sync.dma_start(o[c, :, lo:hi], oc[:])

        def emit_attention():
            gi = 0
            pend = None
            last_key = chunks[-1][0]
            for key, H, qbase, avw, oslice, prs in chunks:
                av = psav.tile([80, avw], F32, tag="av")
                npairs = len(prs)
                split = key == last_key
                for pidx, (g0, off, w, masked) in enumerate(prs):
                    st = psst.tile([128, 2 * H], F32, tag="st")
                    nc.tensor.matmul(
                        st[:, H - w:H], kslot(g0), qsl(qbase + off, w),
                        start=True, stop=True, perf_mode=DR,
                    )
                    nc.tensor.matmul(
                        st[:, H:H + w], kslot(g0 + 1), qsl(qbase + off, w),
                        start=True, stop=True, perf_mode=DR,
                    )
                    for fn in fillers.get(gi, ()):
                        fn()
                    if pend is not None:
                        emit_av(pend)
                        pend = None
                    bf16 = (key == "0a" and pidx == 0)
                    pt = ptp.tile([128, 2 * H], BF16 if bf16 else FP8)
                    nc.scalar.activation(pt[:, H - w:H + w], st[:, H - w:H + w],
                                         EXPF, bias=0.0, scale=SCALE)
                    if masked:
                        nc.vector.tensor_mul(
                            pt[:, H:H + w], pt[:, H:H + w], mask_sb[:, 0:w]
                        )
                    pend = (av, off, w, pt, H, pidx, npairs, bf16,
                            oslice if (pidx == npairs - 1 and not split)
                            else None, g0, split)
                    gi += 1
            emit_av(pend)

        for _rep in range(repeats):
            passA(0)
            passB_kv(0)
            emit_attention()

    nc.compile()
    return nc


def _get_nc():
    global _CACHED_NC
    if _CACHED_NC is None:
        _CACHED_NC = build_nc()
    return _CACHED_NC


def _host_inputs(x, wq, bq, wk, bk, wv, bv):
    bf = ml_dtypes.bfloat16
    # weights pre-arranged to the on-chip [p, (cc, m)] layout so the DMA
    # moves large contiguous runs
    wqr = wq.reshape(NCC, 128, 64).transpose(1, 0, 2).reshape(128, NCC * 64)
    wkv = np.concatenate([wv, wk], axis=1)
    wkvr = wkv.reshape(NCC, 128, 128).transpose(1, 0, 2).reshape(128, NCC * 128)
    wcmb = np.ascontiguousarray(
        np.concatenate([wqr, wkvr], axis=1)).astype(bf)
    tri = np.triu(np.ones((128, 128), np.float32))
    maska = np.concatenate([tri, np.ones((128, 384), np.float32)], axis=1)
    idnb = np.concatenate(
        [np.eye(64, dtype=np.float32), np.zeros((64, 64), np.float32)], axis=0)
    mcmb = np.ascontiguousarray(
        np.concatenate([maska, idnb], axis=1)).astype(bf)
    bcol = np.concatenate([bv, bq])[:, None].astype(np.float32)
    xbf = np.ascontiguousarray(x).astype(bf)

    in_maps = []
    for core in range(8):
        b, j = core // 2, core % 2
        if j == 0:
            xdev = np.concatenate(
                [np.zeros((128, DIN), bf), xbf[b][: SEQ - 128]], axis=0
            )
            ps = np.zeros((128, 1), np.float32)
        else:
            xdev = xbf[b]
            ps = np.ones((128, 1), np.float32)
        # permute each 512-col chunk of x^T to [blocks 1,3 | blocks 0,2]
        xtp = xdev.T.reshape(DIN, NSC, 4, 128)[:, :, [1, 3, 0, 2], :]
        bcmb = np.concatenate([bcol, ps], axis=1).astype(np.float32)
        in_maps.append({
            "xt": np.ascontiguousarray(xtp.reshape(DIN, SEQ)),
            "wcmb": wcmb, "bcmb": np.ascontiguousarray(bcmb),
            "mcmb": mcmb,
        })
    return in_maps


def _assemble(results):
    out = np.empty((4, SEQ, DOUT), np.float32)
    for core in range(8):
        b, j = core // 2, core % 2
        od = results[core]["o"]  # [NQC, 65, 512]
        for c in range(NQC):
            num = od[c, 0:64, :].astype(np.float64)
            den = od[c, 64, :].astype(np.float64)
            oc = (num / den).T.astype(np.float32)  # [512, 64]
            for t in range(4):
                r0 = (8 * c + 2 * t + j) * 128
                out[b, r0:r0 + 128] = oc[t * 128:(t + 1) * 128]
    return out


def kernel(x, wq, bq, wk, bk, wv, bv):
    x = np.asarray(x, dtype=np.float32)
    args = [np.asarray(a, dtype=np.float32) for a in (wq, bq, wk, bk, wv, bv)]
    nc = _get_nc()
    in_maps = _host_inputs(x, *args)
    br = run_bass_kernel_spmd(nc, in_maps, core_ids=list(range(8)))
    return _assemble(br.results)
